# revision 1
# baseline (speedup 1.0000x reference)
"""2-layer GCN on 8 Trainium2 NeuronCores.

Distribution: nodes are range-sharded across 8 cores (dst/graph parallel).
Each core builds its shard of the projected+normalized feature table,
AllGathers the full table, then aggregates messages for edges whose
destination lies in its shard: per destination-tile of 128 nodes, source
rows are fetched with gpsimd dma_gather (int16 indices, 4 address buckets)
and segment-summed via one-hot matmuls accumulating in PSUM.

out[d] = dinv[d] * (sum_{s->d} t[s]) (+ bias), t[s] = (h[s] @ W) * dinv[s],
with self-loops folded into the edge list.
"""
import os
import sys

sys.path.insert(0, "/opt/trn_rl_repo")

import numpy as np

import concourse.bass as bass
import concourse.bacc as bacc
import concourse.tile as tile
import concourse.mybir as mybir
from concourse import bass_utils
from concourse.library_config import mlp

N_CORES = 8
N_NODES = 100000
D_IN, D_H, D_OUT = 128, 64, 64
NSHARD = N_NODES // N_CORES          # 12500
TILE = 128
NT = (NSHARD + TILE - 1) // TILE     # 98
PADN = NT * TILE                     # 12544
N_BUCKET = 4
BUCKET = 2 * PADN                    # 25088 rows per int16 address bucket
PADN_ALL = N_CORES * PADN            # 100352

LAST_RESULT = None
DBG_NO_COLLECTIVE = False   # replace AllGather with local copy (wrong results)
DBG_NO_GATHER = False       # skip dma_gather instructions (wrong results)


def _host_prep(x, edge_index):
    src = np.asarray(edge_index[0], dtype=np.int64)
    dst = np.asarray(edge_index[1], dtype=np.int64)
    n = N_NODES

    deg = np.bincount(dst, minlength=n).astype(np.float64) + 1.0
    dinv = (1.0 / np.sqrt(deg)).astype(np.float32)

    loops = np.arange(n, dtype=np.int64)
    s_all = np.concatenate([src, loops])
    d_all = np.concatenate([dst, loops])

    core = d_all // NSHARD
    drem = d_all % NSHARD
    t_id = drem // TILE
    dloc = drem % TILE
    gsrc = (s_all // NSHARD) * PADN + (s_all % NSHARD)  # padded-global row
    bkt = gsrc // BUCKET
    rel = (gsrc % BUCKET).astype(np.int64)

    key = ((core * NT + t_id) * N_BUCKET + bkt).astype(np.int64)
    order = np.argsort(key, kind="stable")
    key_s = key[order]
    rel_s = rel[order]
    dloc_s = dloc[order]

    ngroups = N_CORES * NT * N_BUCKET
    counts = np.bincount(key_s, minlength=ngroups).reshape(N_CORES, NT, N_BUCKET)
    # chunk count per (tile, bucket), shared across cores (SPMD: one program)
    nb = np.ceil(counts.max(axis=0) / 128.0).astype(np.int64)  # [NT, N_BUCKET]
    nc_t = nb.sum(axis=1)  # chunks per tile [NT]
    chunk_base = np.zeros((NT, N_BUCKET), np.int64)  # chunk-col base of (t,b)
    flat = nb.reshape(-1)
    cb = np.concatenate([[0], np.cumsum(flat)[:-1]]).reshape(NT, N_BUCKET)
    chunk_base = cb
    CHC = int(flat.sum())             # total chunk-cols per core
    IDXC = CHC * 8                    # int16 idx cols (128 = 16 lanes x 8 reps)

    # rank of each edge within its (core,t,b) group
    grp_start = np.zeros(ngroups + 1, np.int64)
    np.cumsum(counts.reshape(-1), out=grp_start[1:])
    rank = np.arange(key_s.shape[0], dtype=np.int64) - grp_start[key_s]

    core_s = key_s // (NT * N_BUCKET)
    tb = key_s % (NT * N_BUCKET)
    t_s = tb // N_BUCKET
    b_s = tb % N_BUCKET

    ccol = chunk_base[t_s, b_s] + rank // 128       # chunk col of edge
    cpart = rank % 128                               # partition of edge

    # dstloc array [cores, 128, CHC], pad 999 -> one-hot never matches
    dstloc = np.full((N_CORES, 128, CHC), 999.0, np.float32)
    dstloc[core_s, cpart, ccol] = dloc_s.astype(np.float32)

    # idx16 wrapped [cores, 128, IDXC]: within (t,b) slice of width nb*8,
    # idx j -> row (j%16 + 16*r for all r), col j//16. pad -1 (skipped).
    idx16_16 = np.zeros((N_CORES, 16, IDXC), np.int16)
    icolbase = chunk_base * 8
    icol = icolbase[t_s, b_s] + rank // 16
    irow = rank % 16
    idx16_16[core_s, irow, icol] = rel_s.astype(np.int16)
    idx16 = np.tile(idx16_16, (1, 8, 1))  # [cores, 128, IDXC]

    # per-core dinv columns [cores, 128, NT] (pad rows -> 0)
    dinv_cols = np.zeros((N_CORES, 128, NT), np.float32)
    node_grid = (
        np.arange(N_CORES)[:, None, None] * NSHARD
        + np.arange(NT)[None, None, :] * TILE
        + np.arange(128)[None, :, None]
    )
    local = (
        np.arange(NT)[None, None, :] * TILE + np.arange(128)[None, :, None]
    )
    valid = np.broadcast_to(local < NSHARD, node_grid.shape)
    node_clip = np.where(valid, node_grid, 0)
    dinv_cols[:] = np.where(valid, dinv[node_clip], 0.0)

    # x shards [cores, PADN, D_IN]
    x_sh = np.zeros((N_CORES, PADN, D_IN), np.float32)
    x_sh[:, :NSHARD] = np.asarray(x, np.float32).reshape(N_CORES, NSHARD, D_IN)

    meta = dict(nb=nb, nc_t=nc_t, chunk_base=chunk_base, CHC=CHC, IDXC=IDXC)
    return x_sh, idx16, dstloc, dinv_cols, meta


def _build_program(meta):
    nb = meta["nb"]
    nc_t = meta["nc_t"]
    chunk_base = meta["chunk_base"]
    CHC, IDXC = meta["CHC"], meta["IDXC"]
    NCMAX = int(nc_t.max())

    f32 = mybir.dt.float32
    nc = bacc.Bacc("TRN2", target_bir_lowering=False, debug=False,
                   num_devices=N_CORES)

    x_in = nc.dram_tensor("x_sh", [PADN, D_IN], f32, kind="ExternalInput").ap()
    w1_in = nc.dram_tensor("W1", [D_IN, D_H], f32, kind="ExternalInput").ap()
    w2_in = nc.dram_tensor("W2", [D_H, D_OUT], f32, kind="ExternalInput").ap()
    b1_in = nc.dram_tensor("b1r", [128, D_H], f32, kind="ExternalInput").ap()
    b2_in = nc.dram_tensor("b2r", [128, D_OUT], f32, kind="ExternalInput").ap()
    id_in = nc.dram_tensor("ident", [128, 128], f32, kind="ExternalInput").ap()
    io_in = nc.dram_tensor("iota", [128, 128], f32, kind="ExternalInput").ap()
    dv_in = nc.dram_tensor("dinv_cols", [128, NT], f32, kind="ExternalInput").ap()
    ix_in = nc.dram_tensor("idx16", [128, IDXC], mybir.dt.int16,
                           kind="ExternalInput").ap()
    dl_in = nc.dram_tensor("dstloc", [128, CHC], f32, kind="ExternalInput").ap()
    out_t = nc.dram_tensor("out", [PADN, D_OUT], f32, kind="ExternalOutput").ap()

    rg = [list(range(N_CORES))]

    with tile.TileContext(nc) as tc:
        with tc.tile_pool(name="const", bufs=1) as constp, \
             tc.tile_pool(name="dram", bufs=1, space="DRAM") as dram, \
             tc.tile_pool(name="xin", bufs=3) as xin, \
             tc.tile_pool(name="tp", bufs=2, space="PSUM") as tpp, \
             tc.tile_pool(name="proj", bufs=2, space="PSUM") as projp, \
             tc.tile_pool(name="agg", bufs=3, space="PSUM") as aggp, \
             tc.tile_pool(name="sb", bufs=3) as sb, \
             tc.tile_pool(name="gat", bufs=3) as gatp, \
             tc.tile_pool(name="sel", bufs=2) as selp, \
             tc.tile_pool(name="meta", bufs=3) as metap:

            nc.gpsimd.load_library(mlp)

            w1 = constp.tile([D_IN, D_H], f32)
            nc.sync.dma_start(w1[:], w1_in[:])
            w2 = constp.tile([D_H, D_OUT], f32)
            nc.sync.dma_start(w2[:], w2_in[:])
            b1r = constp.tile([128, D_H], f32)
            nc.sync.dma_start(b1r[:], b1_in[:])
            b2r = constp.tile([128, D_OUT], f32)
            nc.sync.dma_start(b2r[:], b2_in[:])
            ident = constp.tile([128, 128], f32)
            nc.sync.dma_start(ident[:], id_in[:])
            iota = constp.tile([128, 128], f32)
            nc.sync.dma_start(iota[:], io_in[:])
            dvc = constp.tile([128, NT], f32)
            nc.sync.dma_start(dvc[:], dv_in[:])

            t1_shard = dram.tile([PADN, D_H], f32)
            t1_full = dram.tile([PADN_ALL, D_H], f32)
            t2_shard = dram.tile([PADN, D_H], f32)
            t2_full = dram.tile([PADN_ALL, D_H], f32)

            # ---- phase T1: t1_shard = (x @ W1) * dinv ----
            for t in range(NT):
                xt = xin.tile([128, D_IN], f32, tag="xt")
                nc.sync.dma_start(xt[:], x_in[t * 128:(t + 1) * 128, :])
                xT_ps = tpp.tile([D_IN, 128], f32, tag="tp")
                nc.tensor.transpose(xT_ps[:], xt[:], ident[:])
                xT = sb.tile([D_IN, 128], f32, tag="xT")
                nc.scalar.copy(xT[:], xT_ps[:])
                p1 = projp.tile([128, D_H], f32, tag="proj")
                nc.tensor.matmul(p1[:], lhsT=xT[:], rhs=w1[:],
                                 start=True, stop=True)
                t1t = sb.tile([128, D_H], f32, tag="ev")
                nc.vector.tensor_scalar_mul(t1t[:], p1[:], dvc[:, t:t + 1])
                nc.sync.dma_start(t1_shard[t * 128:(t + 1) * 128, :], t1t[:])

            # ---- AllGather t1 ----
            if DBG_NO_COLLECTIVE:
                nc.sync.dma_start(t1_full[0:PADN, :], t1_shard[:])
            else:
                nc.gpsimd.collective_compute(
                    "AllGather", mybir.AluOpType.bypass,
                    ins=[t1_shard.opt()], outs=[t1_full.opt()],
                    replica_groups=rg,
                )

            def aggregate_layer(table_full, layer):
                btabs = []
                for b in range(N_BUCKET):
                    bt = dram.tile([BUCKET, D_H], f32, tag=f"bt{layer}_{b}")
                    nc.sync.dma_start(
                        bt[:], table_full[b * BUCKET:(b + 1) * BUCKET, :])
                    btabs.append(bt)
                """Per dst tile: gather + one-hot matmul accumulate.
                Returns nothing; evicts per-tile via `evict(t, agg_psum)`."""
                for t in range(NT):
                    NC = int(nc_t[t])
                    icol0 = int(chunk_base[t, 0] * 8)
                    icoln = int((chunk_base[t, 0] + nb[t].sum()) * 8)
                    ccol0 = int(chunk_base[t, 0])

                    it = metap.tile([128, icoln - icol0], mybir.dt.int16,
                                    tag="it")
                    nc.sync.dma_start(it[:], ix_in[:, icol0:icoln])
                    dl = metap.tile([128, NC], f32, tag="dl")
                    nc.sync.dma_start(dl[:], dl_in[:, ccol0:ccol0 + NC])

                    G = gatp.tile([128, NC, D_H], f32, tag="G")
                    nc.vector.memset(G[:], 0.0)
                    for b in range(N_BUCKET):
                        nbb = int(nb[t, b])
                        if nbb == 0:
                            continue
                        cb = int(chunk_base[t, b]) - ccol0
                        tb = btabs[b][:]
                        if not DBG_NO_GATHER:
                            nc.gpsimd.dma_gather(
                                G[:, cb:cb + nbb, :], tb,
                                it[:, cb * 8:(cb + nbb) * 8],
                                nbb * 128, nbb * 128, D_H,
                                single_packet=False,
                            )

                    S = selp.tile([128, NC, 128], f32, tag="S")
                    nc.vector.tensor_tensor(
                        out=S[:],
                        in0=dl[:].to_broadcast([128, NC, 128]),
                        in1=iota[:].unsqueeze(1).to_broadcast([128, NC, 128]),
                        op=mybir.AluOpType.is_equal,
                    )

                    agg = aggp.tile([128, D_H], f32, tag="agg")
                    for c in range(NC):
                        nc.tensor.matmul(
                            agg[:], lhsT=S[:, c, :], rhs=G[:, c, :],
                            start=(c == 0), stop=(c == NC - 1),
                        )

                    if layer == 1:
                        # h = relu(dinv*agg + b1); t2 = (h @ W2) * dinv
                        hv = sb.tile([128, D_H], f32, tag="ev")
                        nc.vector.tensor_scalar_mul(hv[:], agg[:],
                                                    dvc[:, t:t + 1])
                        hb = sb.tile([128, D_H], f32, tag="ev2")
                        nc.vector.tensor_add(hb[:], hv[:], b1r[:])
                        hr = sb.tile([128, D_H], f32, tag="ev3")
                        nc.scalar.activation(hr[:], hb[:],
                                             mybir.ActivationFunctionType.Relu)
                        hT_ps = tpp.tile([D_H, 128], f32, tag="tp")
                        nc.tensor.transpose(hT_ps[:], hr[:], ident[:])
                        hT = sb.tile([D_H, 128], f32, tag="hT")
                        nc.scalar.copy(hT[:], hT_ps[:])
                        p2 = projp.tile([128, D_OUT], f32, tag="proj")
                        nc.tensor.matmul(p2[:], lhsT=hT[:], rhs=w2[:],
                                         start=True, stop=True)
                        t2t = sb.tile([128, D_OUT], f32, tag="ev4")
                        nc.vector.tensor_scalar_mul(t2t[:], p2[:],
                                                    dvc[:, t:t + 1])
                        nc.sync.dma_start(
                            t2_shard[t * 128:(t + 1) * 128, :], t2t[:])
                    else:
                        ov = sb.tile([128, D_OUT], f32, tag="ev")
                        nc.vector.tensor_scalar_mul(ov[:], agg[:],
                                                    dvc[:, t:t + 1])
                        ob = sb.tile([128, D_OUT], f32, tag="ev2")
                        nc.vector.tensor_add(ob[:], ov[:], b2r[:])
                        nc.sync.dma_start(
                            out_t[t * 128:(t + 1) * 128, :], ob[:])

            # ---- layer 1 aggregate + t2 build ----
            aggregate_layer(t1_full[:], layer=1)

            # ---- AllGather t2 ----
            if DBG_NO_COLLECTIVE:
                nc.sync.dma_start(t2_full[0:PADN, :], t2_shard[:])
            else:
                nc.gpsimd.collective_compute(
                    "AllGather", mybir.AluOpType.bypass,
                    ins=[t2_shard.opt()], outs=[t2_full.opt()],
                    replica_groups=rg,
                )

            # ---- layer 2 aggregate -> output ----
            aggregate_layer(t2_full[:], layer=2)

    nc.compile()
    return nc


def kernel(x, edge_index, W1, b1, W2, b2):
    global LAST_RESULT
    x = np.asarray(x, np.float32)
    W1 = np.asarray(W1, np.float32)
    W2 = np.asarray(W2, np.float32)
    b1 = np.asarray(b1, np.float32)
    b2 = np.asarray(b2, np.float32)

    x_sh, idx16, dstloc, dinv_cols, meta = _host_prep(x, edge_index)
    nc = _build_program(meta)

    ident = np.eye(128, dtype=np.float32)
    iota = np.tile(np.arange(128, dtype=np.float32), (128, 1))
    b1r = np.tile(b1[None, :], (128, 1)).astype(np.float32)
    b2r = np.tile(b2[None, :], (128, 1)).astype(np.float32)

    in_maps = []
    for k in range(N_CORES):
        in_maps.append({
            "x_sh": x_sh[k],
            "W1": W1, "W2": W2, "b1r": b1r, "b2r": b2r,
            "ident": ident, "iota": iota,
            "dinv_cols": dinv_cols[k],
            "idx16": idx16[k],
            "dstloc": dstloc[k],
        })

    trace = bool(os.environ.get("BASS_TRACE"))
    res = bass_utils.run_bass_kernel_spmd(
        nc, in_maps, core_ids=list(range(N_CORES)), trace=trace)
    LAST_RESULT = res

    out = np.empty((N_NODES, D_OUT), np.float32)
    for k in range(N_CORES):
        out[k * NSHARD:(k + 1) * NSHARD] = res.results[k]["out"][:NSHARD]
    return out



# revision 2
# speedup vs baseline: 1.1046x; 1.1046x over previous
"""2-layer GCN on 8 Trainium2 NeuronCores — aggregate-then-project.

Nodes are range-sharded across 8 cores (dst parallel). Both GCN layers are
computed as: gather source rows from a bf16 feature table (dma_gather with
2 src buckets, biased signed-int16 indices), segment-sum via one-hot bf16
matmuls into PSUM, then per-dst-tile projection:

  layer1 table = dinv*x (host-precomputed, full -> no collective needed)
      aggx[d]  = sum_{s->d} table1[s]          (self-loops in edge list)
      t2[d]    = dinv[d] * relu(dinv[d]*(aggx[d] @ W1) + b1)
  AllGather t2 -> table2
      out[d]   = dinv[d]*(agg2[d] @ W2) + b2

Gathers are batched per group of 7 dst tiles (2 per group: one per bucket)
to amortize the ~5.5us fixed cost per dma_gather instruction.
"""
import os
import sys

sys.path.insert(0, "/opt/trn_rl_repo")

import numpy as np
import ml_dtypes

import concourse.bass as bass
import concourse.bacc as bacc
import concourse.tile as tile
import concourse.mybir as mybir
from concourse import bass_utils
from concourse.library_config import mlp

N_CORES = 8
N_NODES = 100000
D_IN, D_H, D_OUT = 128, 64, 64
NSHARD = N_NODES // N_CORES          # 12500
TILE = 128
NT = (NSHARD + TILE - 1) // TILE     # 98
PADN = NT * TILE                     # 12544
PADN_ALL = N_CORES * PADN            # 100352
N_BUCKET = 2
B0_ROWS = 65536
BIAS = (32768, B0_ROWS + 32768)
GROUP = 7
N_GROUP = NT // GROUP                # 14

LAST_RESULT = None


def _host_prep(x, edge_index):
    src = np.asarray(edge_index[0], dtype=np.int64)
    dst = np.asarray(edge_index[1], dtype=np.int64)
    n = N_NODES

    deg = np.bincount(dst, minlength=n).astype(np.float64) + 1.0
    dinv = (1.0 / np.sqrt(deg)).astype(np.float32)

    loops = np.arange(n, dtype=np.int64)
    s_all = np.concatenate([src, loops])
    d_all = np.concatenate([dst, loops])

    core = d_all // NSHARD
    drem = d_all % NSHARD
    t_id = drem // TILE
    dloc = drem % TILE
    gsrc = (s_all // NSHARD) * PADN + (s_all % NSHARD)
    bkt = (gsrc >= B0_ROWS).astype(np.int64)

    key = (core * NT + t_id) * N_BUCKET + bkt
    order = np.argsort(key, kind="stable")
    key_s = key[order]
    gsrc_s = gsrc[order]
    dloc_s = dloc[order]

    ngroups = N_CORES * NT * N_BUCKET
    counts = np.bincount(key_s, minlength=ngroups).reshape(N_CORES, NT, N_BUCKET)
    # floor+1 chunks: every (t,b) ends with >=1 pad slot on every core, so the
    # gather ucode's trailing-negative trim can never drop a real edge.
    nb = counts.max(axis=0) // 128 + 1                  # [NT, N_BUCKET]

    # slot layout: per group of GROUP tiles: all b0 chunks (tile-major), then
    # all b1 chunks -> one contiguous gather dst region per (group, bucket).
    chunk_col = np.zeros((NT, N_BUCKET), np.int64)
    grp_nc = np.zeros(N_GROUP, np.int64)
    grp_base = np.zeros(N_GROUP, np.int64)
    grp_b_off = np.zeros((N_GROUP, N_BUCKET + 1), np.int64)
    pos = 0
    for g in range(N_GROUP):
        t0 = g * GROUP
        grp_base[g] = pos
        for b in range(N_BUCKET):
            grp_b_off[g, b] = pos - grp_base[g]
            for t in range(t0, t0 + GROUP):
                chunk_col[t, b] = pos
                pos += nb[t, b]
        grp_nc[g] = pos - grp_base[g]
        grp_b_off[g, N_BUCKET] = grp_nc[g]
    CHC = pos
    IDXC16 = CHC * 8

    grp_start = np.zeros(ngroups + 1, np.int64)
    np.cumsum(counts.reshape(-1), out=grp_start[1:])
    rank = np.arange(key_s.shape[0], dtype=np.int64) - grp_start[key_s]

    core_s = key_s // (NT * N_BUCKET)
    tb = key_s % (NT * N_BUCKET)
    t_s = tb // N_BUCKET
    b_s = tb % N_BUCKET

    slot = chunk_col[t_s, b_s] * 128 + rank
    ccol = slot // 128
    cpart = slot % 128

    idx_val = gsrc_s - np.where(b_s == 0, BIAS[0], BIAS[1])
    assert idx_val.min() >= -32768 and idx_val.max() <= 32767

    idx16_16 = np.zeros((N_CORES, 16, IDXC16), np.int16)
    idx16_16[core_s, slot % 16, slot // 16] = idx_val.astype(np.int16)
    idx16 = np.tile(idx16_16, (1, 8, 1))                # [cores, 128, IDXC16]

    dstloc = np.full((N_CORES, 128, CHC), 999.0, np.float32)
    dstloc[core_s, cpart, ccol] = dloc_s.astype(np.float32)
    dstloc = dstloc.astype(ml_dtypes.bfloat16)

    dinv_cols = np.zeros((N_CORES, 128, NT), np.float32)
    node_grid = (
        np.arange(N_CORES)[:, None, None] * NSHARD
        + np.arange(NT)[None, None, :] * TILE
        + np.arange(128)[None, :, None]
    )
    local = np.arange(NT)[None, None, :] * TILE + np.arange(128)[None, :, None]
    valid = np.broadcast_to(local < NSHARD, node_grid.shape)
    dinv_cols[:] = np.where(valid, dinv[np.where(valid, node_grid, 0)], 0.0)

    xt = np.zeros((PADN_ALL, D_IN), np.float32)
    xs = (np.asarray(x, np.float32) * dinv[:, None]).reshape(N_CORES, NSHARD, D_IN)
    xt.reshape(N_CORES, PADN, D_IN)[:, :NSHARD] = xs
    xt = xt.astype(ml_dtypes.bfloat16)

    meta = dict(nb=nb, chunk_col=chunk_col, grp_nc=grp_nc, grp_base=grp_base,
                grp_b_off=grp_b_off, CHC=CHC, IDXC16=IDXC16)
    return xt, idx16, dstloc, dinv_cols, meta


def _build_program(meta):
    nb = meta["nb"]
    chunk_col = meta["chunk_col"]
    grp_nc = meta["grp_nc"]
    grp_base = meta["grp_base"]
    grp_b_off = meta["grp_b_off"]
    CHC, IDXC16 = meta["CHC"], meta["IDXC16"]

    f32 = mybir.dt.float32
    bf16 = mybir.dt.bfloat16
    i16 = mybir.dt.int16
    nc = bacc.Bacc("TRN2", target_bir_lowering=False, debug=False,
                   num_devices=N_CORES)

    xt_in = nc.dram_tensor("xt", [PADN_ALL, D_IN], bf16, kind="ExternalInput").ap()
    w1_in = nc.dram_tensor("W1", [D_IN, D_H], f32, kind="ExternalInput").ap()
    w2_in = nc.dram_tensor("W2", [D_H, D_OUT], f32, kind="ExternalInput").ap()
    b1_in = nc.dram_tensor("b1r", [128, D_H], f32, kind="ExternalInput").ap()
    b2_in = nc.dram_tensor("b2r", [128, D_OUT], f32, kind="ExternalInput").ap()
    id_in = nc.dram_tensor("ident", [128, 128], f32, kind="ExternalInput").ap()
    io_in = nc.dram_tensor("iota", [128, 128], bf16, kind="ExternalInput").ap()
    dv_in = nc.dram_tensor("dinv_cols", [128, NT], f32, kind="ExternalInput").ap()
    ix_in = nc.dram_tensor("idx16", [128, IDXC16], i16, kind="ExternalInput").ap()
    dl_in = nc.dram_tensor("dstloc", [128, CHC], bf16, kind="ExternalInput").ap()
    out_t = nc.dram_tensor("out", [PADN, D_OUT], f32, kind="ExternalOutput").ap()

    rg = [list(range(N_CORES))]

    with tile.TileContext(nc) as tc:
        with tc.tile_pool(name="const", bufs=1) as constp, \
             tc.tile_pool(name="dram", bufs=1, space="DRAM") as dram, \
             tc.tile_pool(name="agg", bufs=2, space="PSUM") as aggp, \
             tc.tile_pool(name="tp", bufs=2, space="PSUM") as tpp, \
             tc.tile_pool(name="proj", bufs=2, space="PSUM") as projp, \
             tc.tile_pool(name="gat", bufs=2) as gatp, \
             tc.tile_pool(name="sel", bufs=2) as selp, \
             tc.tile_pool(name="sb", bufs=3) as sb:

            nc.gpsimd.load_library(mlp)

            w1 = constp.tile([D_IN, D_H], f32)
            nc.sync.dma_start(w1[:], w1_in[:])
            w2 = constp.tile([D_H, D_OUT], f32)
            nc.sync.dma_start(w2[:], w2_in[:])
            b1r = constp.tile([128, D_H], f32)
            nc.sync.dma_start(b1r[:], b1_in[:])
            b2r = constp.tile([128, D_OUT], f32)
            nc.sync.dma_start(b2r[:], b2_in[:])
            ident = constp.tile([128, 128], f32)
            nc.sync.dma_start(ident[:], id_in[:])
            iota = constp.tile([128, 128], bf16)
            nc.sync.dma_start(iota[:], io_in[:])
            dvc = constp.tile([128, NT], f32)
            nc.sync.dma_start(dvc[:], dv_in[:])
            # idx + dstloc resident for both layers
            ixall = constp.tile([128, IDXC16], i16)
            nc.sync.dma_start(ixall[:], ix_in[:])
            dlall = constp.tile([128, CHC], bf16)
            nc.sync.dma_start(dlall[:], dl_in[:])

            t2_shard = dram.tile([PADN, D_IN], bf16)
            t2_full = dram.tile([PADN_ALL, D_IN], bf16)

            def edge_pass(table_ap, width, layer):
                """table_ap: [PADN_ALL, 128] bf16 AP. width: 128 or 64."""
                tb0 = table_ap[BIAS[0]:B0_ROWS, :]
                tb1 = table_ap[BIAS[1]:PADN_ALL, :]
                for g in range(N_GROUP):
                    base = int(grp_base[g])
                    ncg = int(grp_nc[g])
                    G = gatp.tile([128, ncg, D_IN], bf16, tag="G")
                    for b in range(N_BUCKET):
                        off = int(grp_b_off[g, b])
                        ncb = int(grp_b_off[g, b + 1]) - off
                        nidx = ncb * 128
                        nc.gpsimd.dma_gather(
                            G[:, off:off + ncb, :],
                            tb0 if b == 0 else tb1,
                            ixall[:, (base + off) * 8:(base + off + ncb) * 8],
                            nidx, nidx, D_IN,
                            single_packet=False,
                        )
                    for t in range(g * GROUP, (g + 1) * GROUP):
                        nb0 = int(nb[t, 0])
                        nb1 = int(nb[t, 1])
                        nct = nb0 + nb1
                        l0 = int(chunk_col[t, 0]) - base
                        l1 = int(chunk_col[t, 1]) - base
                        S = selp.tile([128, nct, 128], bf16, tag="S")
                        nc.vector.tensor_tensor(
                            out=S[:, 0:nb0, :],
                            in0=dlall[:, base + l0:base + l0 + nb0]
                                .to_broadcast([128, nb0, 128]),
                            in1=iota[:].unsqueeze(1).to_broadcast([128, nb0, 128]),
                            op=mybir.AluOpType.is_equal,
                        )
                        nc.vector.tensor_tensor(
                            out=S[:, nb0:nct, :],
                            in0=dlall[:, base + l1:base + l1 + nb1]
                                .to_broadcast([128, nb1, 128]),
                            in1=iota[:].unsqueeze(1).to_broadcast([128, nb1, 128]),
                            op=mybir.AluOpType.is_equal,
                        )
                        agg = aggp.tile([128, width], f32, tag="agg")
                        for i in range(nct):
                            gc = (l0 + i) if i < nb0 else (l1 + i - nb0)
                            nc.tensor.matmul(
                                agg[:], lhsT=S[:, i, :],
                                rhs=G[:, gc, 0:width],
                                start=(i == 0), stop=(i == nct - 1),
                            )
                        if layer == 1:
                            # t2 = dinv*relu(dinv*(agg @ W1) + b1)
                            preS = sb.tile([128, 128], f32, tag="e1")
                            nc.scalar.copy(preS[:], agg[:])
                            preT_ps = tpp.tile([128, 128], f32, tag="tp")
                            nc.tensor.transpose(preT_ps[:], preS[:], ident[:])
                            preT = sb.tile([128, 128], f32, tag="e2")
                            nc.scalar.copy(preT[:], preT_ps[:])
                            proj = projp.tile([128, D_H], f32, tag="proj")
                            nc.tensor.matmul(proj[:], lhsT=preT[:], rhs=w1[:],
                                             start=True, stop=True)
                            hv = sb.tile([128, D_H], f32, tag="e3")
                            nc.vector.tensor_scalar_mul(hv[:], proj[:],
                                                        dvc[:, t:t + 1])
                            hb = sb.tile([128, D_H], f32, tag="e4")
                            nc.vector.tensor_add(hb[:], hv[:], b1r[:])
                            hr = sb.tile([128, D_H], f32, tag="e5")
                            nc.scalar.activation(
                                hr[:], hb[:], mybir.ActivationFunctionType.Relu)
                            t2t = sb.tile([128, D_H], bf16, tag="e6")
                            nc.vector.tensor_scalar_mul(t2t[:], hr[:],
                                                        dvc[:, t:t + 1])
                            nc.sync.dma_start(
                                t2_shard[t * 128:(t + 1) * 128, 0:D_H], t2t[:])
                        else:
                            # out = dinv*(agg @ W2) + b2
                            a2S = sb.tile([128, D_H], f32, tag="e1")
                            nc.scalar.copy(a2S[:], agg[:])
                            a2T_ps = tpp.tile([D_H, 128], f32, tag="tp")
                            nc.tensor.transpose(a2T_ps[:], a2S[:], ident[:])
                            a2T = sb.tile([D_H, 128], f32, tag="e2")
                            nc.scalar.copy(a2T[:], a2T_ps[:])
                            o_ps = projp.tile([128, D_OUT], f32, tag="proj")
                            nc.tensor.matmul(o_ps[:], lhsT=a2T[:], rhs=w2[:],
                                             start=True, stop=True)
                            ov = sb.tile([128, D_OUT], f32, tag="e3")
                            nc.vector.tensor_scalar_mul(ov[:], o_ps[:],
                                                        dvc[:, t:t + 1])
                            ob = sb.tile([128, D_OUT], f32, tag="e4")
                            nc.vector.tensor_add(ob[:], ov[:], b2r[:])
                            nc.sync.dma_start(
                                out_t[t * 128:(t + 1) * 128, :], ob[:])

            edge_pass(xt_in, 128, layer=1)

            nc.gpsimd.collective_compute(
                "AllGather", mybir.AluOpType.bypass,
                ins=[t2_shard.opt()], outs=[t2_full.opt()],
                replica_groups=rg,
            )

            edge_pass(t2_full[:], D_H, layer=2)

    nc.compile()
    return nc


def kernel(x, edge_index, W1, b1, W2, b2):
    global LAST_RESULT
    x = np.asarray(x, np.float32)
    W1 = np.asarray(W1, np.float32)
    W2 = np.asarray(W2, np.float32)
    b1 = np.asarray(b1, np.float32)
    b2 = np.asarray(b2, np.float32)

    xt, idx16, dstloc, dinv_cols, meta = _host_prep(x, edge_index)
    nc = _build_program(meta)

    ident = np.eye(128, dtype=np.float32)
    iota = np.tile(np.arange(128, dtype=np.float32), (128, 1)).astype(
        ml_dtypes.bfloat16)
    b1r = np.tile(b1[None, :], (128, 1)).astype(np.float32)
    b2r = np.tile(b2[None, :], (128, 1)).astype(np.float32)

    in_maps = []
    for k in range(N_CORES):
        in_maps.append({
            "xt": xt,
            "W1": W1, "W2": W2, "b1r": b1r, "b2r": b2r,
            "ident": ident, "iota": iota,
            "dinv_cols": dinv_cols[k],
            "idx16": idx16[k],
            "dstloc": dstloc[k],
        })

    trace = bool(os.environ.get("BASS_TRACE"))
    res = bass_utils.run_bass_kernel_spmd(
        nc, in_maps, core_ids=list(range(N_CORES)), trace=trace)
    LAST_RESULT = res

    out = np.empty((N_NODES, D_OUT), np.float32)
    for k in range(N_CORES):
        out[k * NSHARD:(k + 1) * NSHARD] = res.results[k]["out"][:NSHARD]
    return out


# revision 8
# speedup vs baseline: 1.8646x; 1.6880x over previous
"""2-layer GCN on 8 Trainium2 NeuronCores — aggregate-then-project.

Nodes are range-sharded across 8 cores (dst parallel). Both GCN layers are
computed as: gather source rows from a bf16 feature table (dma_gather with
2 src buckets, biased signed-int16 indices), segment-sum via one-hot bf16
matmuls into PSUM, then per-dst-tile projection:

  layer1 table = dinv*x (host-precomputed, full -> no collective needed)
      aggx[d]  = sum_{s->d} table1[s]          (self-loops in edge list)
      t2[d]    = dinv[d] * relu(dinv[d]*(aggx[d] @ W1) + b1)
  AllGather t2 -> table2
      out[d]   = dinv[d]*(agg2[d] @ W2) + b2

Gathers are batched per group of 7 dst tiles (2 per group: one per bucket)
to amortize the ~5.5us fixed cost per dma_gather instruction.
"""
import os
import sys

sys.path.insert(0, "/opt/trn_rl_repo")

import numpy as np
import ml_dtypes

import concourse.bass as bass
import concourse.bacc as bacc
import concourse.tile as tile
import concourse.mybir as mybir
from concourse import bass_utils
from concourse.library_config import mlp

N_CORES = 8
N_NODES = 100000
D_IN, D_H, D_OUT = 128, 64, 64
NSHARD = N_NODES // N_CORES          # 12500
TILE = 128
NT = (NSHARD + TILE - 1) // TILE     # 98
PADN = NT * TILE                     # 12544
PADN_ALL = N_CORES * PADN            # 100352
N_BUCKET = 2
B0_ROWS = 65536
BIAS = (32768, B0_ROWS + 32768)
GROUP = 7
N_GROUP = NT // GROUP                # 14

LAST_RESULT = None


def _host_prep(x, edge_index):
    src = np.asarray(edge_index[0], dtype=np.int64)
    dst = np.asarray(edge_index[1], dtype=np.int64)
    n = N_NODES

    deg = np.bincount(dst, minlength=n).astype(np.float64) + 1.0
    dinv = (1.0 / np.sqrt(deg)).astype(np.float32)

    loops = np.arange(n, dtype=np.int64)
    s_all = np.concatenate([src, loops])
    d_all = np.concatenate([dst, loops])

    core = d_all // NSHARD
    drem = d_all % NSHARD
    t_id = drem // TILE
    dloc = drem % TILE
    gsrc = (s_all // NSHARD) * PADN + (s_all % NSHARD)
    bkt = (gsrc >= B0_ROWS).astype(np.int64)

    key = (core * NT + t_id) * N_BUCKET + bkt
    order = np.argsort(key, kind="stable")
    key_s = key[order]
    gsrc_s = gsrc[order]
    dloc_s = dloc[order]

    ngroups = N_CORES * NT * N_BUCKET
    counts = np.bincount(key_s, minlength=ngroups).reshape(N_CORES, NT, N_BUCKET)
    # floor+1 chunks: every (t,b) ends with >=1 pad slot on every core, so the
    # gather ucode's trailing-negative trim can never drop a real edge.
    nb = counts.max(axis=0) // 128 + 1                  # [NT, N_BUCKET]

    # slot layout: per group of GROUP tiles: all b0 chunks (tile-major), then
    # all b1 chunks -> one contiguous gather dst region per (group, bucket).
    chunk_col = np.zeros((NT, N_BUCKET), np.int64)
    grp_nc = np.zeros(N_GROUP, np.int64)
    grp_base = np.zeros(N_GROUP, np.int64)
    grp_b_off = np.zeros((N_GROUP, N_BUCKET + 1), np.int64)
    pos = 0
    for g in range(N_GROUP):
        t0 = g * GROUP
        grp_base[g] = pos
        for b in range(N_BUCKET):
            grp_b_off[g, b] = pos - grp_base[g]
            for t in range(t0, t0 + GROUP):
                chunk_col[t, b] = pos
                pos += nb[t, b]
        grp_nc[g] = pos - grp_base[g]
        grp_b_off[g, N_BUCKET] = grp_nc[g]
    CHC = pos
    IDXC16 = CHC * 8

    grp_start = np.zeros(ngroups + 1, np.int64)
    np.cumsum(counts.reshape(-1), out=grp_start[1:])
    rank = np.arange(key_s.shape[0], dtype=np.int64) - grp_start[key_s]

    core_s = key_s // (NT * N_BUCKET)
    tb = key_s % (NT * N_BUCKET)
    t_s = tb // N_BUCKET
    b_s = tb % N_BUCKET

    slot = chunk_col[t_s, b_s] * 128 + rank
    ccol = slot // 128
    cpart = slot % 128

    idx_val = gsrc_s - np.where(b_s == 0, BIAS[0], BIAS[1])
    assert idx_val.min() >= -32768 and idx_val.max() <= 32767

    idx16_16 = np.zeros((N_CORES, 16, IDXC16), np.int16)
    idx16_16[core_s, slot % 16, slot // 16] = idx_val.astype(np.int16)
    idx16 = np.tile(idx16_16, (1, 8, 1))                # [cores, 128, IDXC16]

    dstloc = np.full((N_CORES, 128, CHC), 999.0, np.float32)
    dstloc[core_s, cpart, ccol] = dloc_s.astype(np.float32)

    dinv_cols = np.zeros((N_CORES, 128, NT), np.float32)
    node_grid = (
        np.arange(N_CORES)[:, None, None] * NSHARD
        + np.arange(NT)[None, None, :] * TILE
        + np.arange(128)[None, :, None]
    )
    local = np.arange(NT)[None, None, :] * TILE + np.arange(128)[None, :, None]
    valid = np.broadcast_to(local < NSHARD, node_grid.shape)
    dinv_cols[:] = np.where(valid, dinv[np.where(valid, node_grid, 0)], 0.0)

    xt = np.zeros((PADN_ALL, D_IN), np.float32)
    xs = (np.asarray(x, np.float32) * dinv[:, None]).reshape(N_CORES, NSHARD, D_IN)
    xt.reshape(N_CORES, PADN, D_IN)[:, :NSHARD] = xs
    xt = xt.astype(ml_dtypes.bfloat16)

    meta = dict(nb=nb, chunk_col=chunk_col, grp_nc=grp_nc, grp_base=grp_base,
                grp_b_off=grp_b_off, CHC=CHC, IDXC16=IDXC16)
    return xt, idx16, dstloc, dinv_cols, meta


def _build_program(meta):
    nb = meta["nb"]
    chunk_col = meta["chunk_col"]
    grp_nc = meta["grp_nc"]
    grp_base = meta["grp_base"]
    grp_b_off = meta["grp_b_off"]
    CHC, IDXC16 = meta["CHC"], meta["IDXC16"]

    f32 = mybir.dt.float32
    bf16 = mybir.dt.bfloat16
    i16 = mybir.dt.int16
    nc = bacc.Bacc("TRN2", target_bir_lowering=False, debug=False,
                   num_devices=N_CORES, num_swdge_queues=4)

    xt_in = nc.dram_tensor("xt", [PADN_ALL, D_IN], bf16, kind="ExternalInput").ap()
    w1_in = nc.dram_tensor("W1", [D_IN, D_H], f32, kind="ExternalInput").ap()
    w2_in = nc.dram_tensor("W2", [D_H, D_OUT], f32, kind="ExternalInput").ap()
    b1_in = nc.dram_tensor("b1r", [128, D_H], f32, kind="ExternalInput").ap()
    b2_in = nc.dram_tensor("b2r", [128, D_OUT], f32, kind="ExternalInput").ap()
    id_in = nc.dram_tensor("ident", [128, 128], f32, kind="ExternalInput").ap()
    io_in = nc.dram_tensor("iota", [128, 128], f32, kind="ExternalInput").ap()
    dv_in = nc.dram_tensor("dinv_cols", [128, NT], f32, kind="ExternalInput").ap()
    ix_in = nc.dram_tensor("idx16", [128, IDXC16], i16, kind="ExternalInput").ap()
    dl_in = nc.dram_tensor("dstloc", [128, CHC], f32, kind="ExternalInput").ap()
    out_t = nc.dram_tensor("out", [PADN, D_OUT], f32, kind="ExternalOutput").ap()

    rg = [list(range(N_CORES))]

    with tile.TileContext(nc) as tc:
        with tc.tile_pool(name="const", bufs=1) as constp, \
             tc.tile_pool(name="dram", bufs=1, space="DRAM") as dram, \
             tc.tile_pool(name="agg", bufs=2, space="PSUM") as aggp, \
             tc.tile_pool(name="tp", bufs=2, space="PSUM") as tpp, \
             tc.tile_pool(name="proj", bufs=2, space="PSUM") as projp, \
             tc.tile_pool(name="gat", bufs=2) as gatp, \
             tc.tile_pool(name="sel", bufs=2) as selp, \
             tc.tile_pool(name="sb", bufs=3) as sb:

            nc.gpsimd.load_library(mlp)

            w1 = constp.tile([D_IN, D_H], f32)
            nc.sync.dma_start(w1[:], w1_in[:])
            w2 = constp.tile([D_H, D_OUT], f32)
            nc.sync.dma_start(w2[:], w2_in[:])
            b1r = constp.tile([128, D_H], f32)
            nc.sync.dma_start(b1r[:], b1_in[:])
            b2r = constp.tile([128, D_OUT], f32)
            nc.sync.dma_start(b2r[:], b2_in[:])
            ident = constp.tile([128, 128], f32)
            nc.sync.dma_start(ident[:], id_in[:])
            iota = constp.tile([128, 128], f32)
            nc.sync.dma_start(iota[:], io_in[:])
            dvc = constp.tile([128, NT], f32)
            nc.sync.dma_start(dvc[:], dv_in[:])
            # idx + dstloc resident for both layers
            ixall = constp.tile([128, IDXC16], i16)
            nc.sync.dma_start(ixall[:], ix_in[:])
            dlall = constp.tile([128, CHC], f32)
            nc.sync.dma_start(dlall[:], dl_in[:])

            t2_shard = dram.tile([PADN, D_IN], bf16)
            t2_full = dram.tile([PADN_ALL, D_IN], bf16)

            def edge_pass(table_ap, width, layer):
                """table_ap: [PADN_ALL, 128] bf16 AP. width: 128 or 64."""
                tb0 = table_ap[BIAS[0]:B0_ROWS, :]
                tb1 = table_ap[BIAS[1]:PADN_ALL, :]
                for g in range(N_GROUP):
                    base = int(grp_base[g])
                    ncg = int(grp_nc[g])
                    G = gatp.tile([128, ncg, D_IN], bf16, tag="G")
                    for b in range(N_BUCKET):
                        off = int(grp_b_off[g, b])
                        ncb = int(grp_b_off[g, b + 1]) - off
                        nidx = ncb * 128
                        nc.gpsimd.dma_gather(
                            G[:, off:off + ncb, :],
                            tb0 if b == 0 else tb1,
                            ixall[:, (base + off) * 8:(base + off + ncb) * 8],
                            nidx, nidx, D_IN,
                            single_packet=False,
                            queue_num=(2 * g + b) % 4,
                        )
                    for t in range(g * GROUP, (g + 1) * GROUP):
                        nb0 = int(nb[t, 0])
                        nb1 = int(nb[t, 1])
                        nct = nb0 + nb1
                        l0 = int(chunk_col[t, 0]) - base
                        l1 = int(chunk_col[t, 1]) - base
                        S0 = selp.tile([128, nb0, 128], bf16, tag="S0")
                        nc.vector.tensor_tensor(
                            out=S0[:],
                            in0=dlall[:, base + l0:base + l0 + nb0]
                                .to_broadcast([128, nb0, 128]),
                            in1=iota[:].unsqueeze(1).to_broadcast([128, nb0, 128]),
                            op=mybir.AluOpType.is_equal,
                        )
                        S1 = selp.tile([128, nb1, 128], bf16, tag="S1")
                        nc.vector.tensor_tensor(
                            out=S1[:],
                            in0=dlall[:, base + l1:base + l1 + nb1]
                                .to_broadcast([128, nb1, 128]),
                            in1=iota[:].unsqueeze(1).to_broadcast([128, nb1, 128]),
                            op=mybir.AluOpType.is_equal,
                        )
                        agg = aggp.tile([128, width], f32, tag="agg")
                        for i in range(nct):
                            S = S0[:, i, :] if i < nb0 else S1[:, i - nb0, :]
                            gc = (l0 + i) if i < nb0 else (l1 + i - nb0)
                            nc.tensor.matmul(
                                agg[:], lhsT=S,
                                rhs=G[:, gc, 0:width],
                                start=(i == 0), stop=(i == nct - 1),
                            )
                        if layer == 1:
                            # t2 = dinv*relu(dinv*(agg @ W1) + b1)
                            preS = sb.tile([128, 128], f32, tag="e1")
                            nc.scalar.copy(preS[:], agg[:])
                            preT_ps = tpp.tile([128, 128], f32, tag="tp")
                            nc.tensor.transpose(preT_ps[:], preS[:], ident[:])
                            preT = sb.tile([128, 128], f32, tag="e2")
                            nc.scalar.copy(preT[:], preT_ps[:])
                            proj = projp.tile([128, D_H], f32, tag="proj")
                            nc.tensor.matmul(proj[:], lhsT=preT[:], rhs=w1[:],
                                             start=True, stop=True)
                            hv = sb.tile([128, D_H], f32, tag="e3")
                            nc.scalar.activation(
                                hv[:], proj[:],
                                mybir.ActivationFunctionType.Copy,
                                scale=dvc[:, t:t + 1])
                            hb = sb.tile([128, D_H], f32, tag="e4")
                            nc.vector.tensor_add(hb[:], hv[:], b1r[:])
                            hr = sb.tile([128, D_H], f32, tag="e5")
                            nc.scalar.activation(
                                hr[:], hb[:], mybir.ActivationFunctionType.Relu)
                            t2t = sb.tile([128, D_H], bf16, tag="e6")
                            nc.scalar.activation(
                                t2t[:], hr[:],
                                mybir.ActivationFunctionType.Copy,
                                scale=dvc[:, t:t + 1])
                            nc.sync.dma_start(
                                t2_shard[t * 128:(t + 1) * 128, 0:D_H], t2t[:])
                        else:
                            # out = dinv*(agg @ W2) + b2
                            a2S = sb.tile([128, D_H], f32, tag="e1")
                            nc.scalar.copy(a2S[:], agg[:])
                            a2T_ps = tpp.tile([D_H, 128], f32, tag="tp")
                            nc.tensor.transpose(a2T_ps[:], a2S[:], ident[:])
                            a2T = sb.tile([D_H, 128], f32, tag="e2")
                            nc.scalar.copy(a2T[:], a2T_ps[:])
                            o_ps = projp.tile([128, D_OUT], f32, tag="proj")
                            nc.tensor.matmul(o_ps[:], lhsT=a2T[:], rhs=w2[:],
                                             start=True, stop=True)
                            ov = sb.tile([128, D_OUT], f32, tag="e3")
                            nc.scalar.activation(
                                ov[:], o_ps[:],
                                mybir.ActivationFunctionType.Copy,
                                scale=dvc[:, t:t + 1])
                            ob = sb.tile([128, D_OUT], f32, tag="e4")
                            nc.vector.tensor_add(ob[:], ov[:], b2r[:])
                            nc.sync.dma_start(
                                out_t[t * 128:(t + 1) * 128, :], ob[:])

            edge_pass(xt_in, 128, layer=1)

            nc.gpsimd.collective_compute(
                "AllGather", mybir.AluOpType.bypass,
                ins=[t2_shard.opt()], outs=[t2_full.opt()],
                replica_groups=rg,
            )

            edge_pass(t2_full[:], D_H, layer=2)

    nc.compile()
    return nc


def kernel(x, edge_index, W1, b1, W2, b2):
    global LAST_RESULT
    x = np.asarray(x, np.float32)
    W1 = np.asarray(W1, np.float32)
    W2 = np.asarray(W2, np.float32)
    b1 = np.asarray(b1, np.float32)
    b2 = np.asarray(b2, np.float32)

    xt, idx16, dstloc, dinv_cols, meta = _host_prep(x, edge_index)
    nc = _build_program(meta)

    ident = np.eye(128, dtype=np.float32)
    iota = np.tile(np.arange(128, dtype=np.float32), (128, 1))
    b1r = np.tile(b1[None, :], (128, 1)).astype(np.float32)
    b2r = np.tile(b2[None, :], (128, 1)).astype(np.float32)

    in_maps = []
    for k in range(N_CORES):
        in_maps.append({
            "xt": xt,
            "W1": W1, "W2": W2, "b1r": b1r, "b2r": b2r,
            "ident": ident, "iota": iota,
            "dinv_cols": dinv_cols[k],
            "idx16": idx16[k],
            "dstloc": dstloc[k],
        })

    trace = bool(os.environ.get("BASS_TRACE"))
    res = bass_utils.run_bass_kernel_spmd(
        nc, in_maps, core_ids=list(range(N_CORES)), trace=trace)
    LAST_RESULT = res

    out = np.empty((N_NODES, D_OUT), np.float32)
    for k in range(N_CORES):
        out[k * NSHARD:(k + 1) * NSHARD] = res.results[k]["out"][:NSHARD]
    return out


# revision 10
# speedup vs baseline: 2.4849x; 1.3327x over previous
"""2-layer GCN on 8 Trainium2 NeuronCores — aggregate-then-project.

Nodes are range-sharded across 8 cores (dst parallel). Both GCN layers are
computed as: gather source rows from a bf16 feature table (dma_gather with
2 src buckets, biased signed-int16 indices), segment-sum via one-hot bf16
matmuls into PSUM, then per-dst-tile projection:

  layer1 table = dinv*x (host-precomputed, full -> no collective needed)
      aggx[d]  = sum_{s->d} table1[s]          (self-loops in edge list)
      t2[d]    = dinv[d] * relu(dinv[d]*(aggx[d] @ W1) + b1)
  AllGather t2 -> table2
      out[d]   = dinv[d]*(agg2[d] @ W2) + b2

Gathers are batched per group of 7 dst tiles (2 per group: one per bucket)
to amortize the ~5.5us fixed cost per dma_gather instruction.
"""
import os
import sys

sys.path.insert(0, "/opt/trn_rl_repo")

import numpy as np
import ml_dtypes

import concourse.bass as bass
import concourse.bacc as bacc
import concourse.tile as tile
import concourse.mybir as mybir
from concourse import bass_utils
from concourse.library_config import mlp

N_CORES = 8
N_NODES = 100000
D_IN, D_H, D_OUT = 128, 64, 64
NSHARD = N_NODES // N_CORES          # 12500
TILE = 128
NT = (NSHARD + TILE - 1) // TILE     # 98
PADN = NT * TILE                     # 12544
PADN_ALL = N_CORES * PADN            # 100352
N_BUCKET = 2
B0_ROWS = 50176
BIAS = (32768, B0_ROWS + 32768)
GROUP = 7
N_GROUP = NT // GROUP                # 14

LAST_RESULT = None


def _host_prep(x, edge_index):
    src = np.asarray(edge_index[0], dtype=np.int64)
    dst = np.asarray(edge_index[1], dtype=np.int64)
    n = N_NODES

    deg = np.bincount(dst, minlength=n).astype(np.float64) + 1.0
    dinv = (1.0 / np.sqrt(deg)).astype(np.float32)

    loops = np.arange(n, dtype=np.int64)
    s_all = np.concatenate([src, loops])
    d_all = np.concatenate([dst, loops])

    core = d_all // NSHARD
    drem = d_all % NSHARD
    t_id = drem // TILE
    dloc = drem % TILE
    gsrc = (s_all // NSHARD) * PADN + (s_all % NSHARD)
    bkt = (gsrc >= B0_ROWS).astype(np.int64)

    key = (core * NT + t_id) * N_BUCKET + bkt
    order = np.argsort(key, kind="stable")
    key_s = key[order]
    gsrc_s = gsrc[order]
    dloc_s = dloc[order]

    ngroups = N_CORES * NT * N_BUCKET
    counts = np.bincount(key_s, minlength=ngroups).reshape(N_CORES, NT, N_BUCKET)
    # floor+1 chunks: every (t,b) ends with >=1 pad slot on every core, so the
    # gather ucode's trailing-negative trim can never drop a real edge.
    nb = counts.max(axis=0) // 128 + 1                  # [NT, N_BUCKET]

    # slot layout: per group of GROUP tiles: all b0 chunks (tile-major), then
    # all b1 chunks -> one contiguous gather dst region per (group, bucket).
    chunk_col = np.zeros((NT, N_BUCKET), np.int64)
    grp_nc = np.zeros(N_GROUP, np.int64)
    grp_base = np.zeros(N_GROUP, np.int64)
    grp_b_off = np.zeros((N_GROUP, N_BUCKET + 1), np.int64)
    pos = 0
    for g in range(N_GROUP):
        t0 = g * GROUP
        grp_base[g] = pos
        for b in range(N_BUCKET):
            grp_b_off[g, b] = pos - grp_base[g]
            for t in range(t0, t0 + GROUP):
                chunk_col[t, b] = pos
                pos += nb[t, b]
        grp_nc[g] = pos - grp_base[g]
        grp_b_off[g, N_BUCKET] = grp_nc[g]
    CHC = pos
    IDXC16 = CHC * 8

    grp_start = np.zeros(ngroups + 1, np.int64)
    np.cumsum(counts.reshape(-1), out=grp_start[1:])
    rank = np.arange(key_s.shape[0], dtype=np.int64) - grp_start[key_s]

    core_s = key_s // (NT * N_BUCKET)
    tb = key_s % (NT * N_BUCKET)
    t_s = tb // N_BUCKET
    b_s = tb % N_BUCKET

    slot = chunk_col[t_s, b_s] * 128 + rank
    ccol = slot // 128
    cpart = slot % 128

    idx_val = gsrc_s - np.where(b_s == 0, BIAS[0], BIAS[1])
    assert idx_val.min() >= -32768 and idx_val.max() <= 32767

    idx16_16 = np.zeros((N_CORES, 16, IDXC16), np.int16)
    idx16_16[core_s, slot % 16, slot // 16] = idx_val.astype(np.int16)
    idx16 = np.tile(idx16_16, (1, 8, 1))                # [cores, 128, IDXC16]

    dstloc = np.full((N_CORES, 128, CHC), 999.0, np.float32)
    dstloc[core_s, cpart, ccol] = dloc_s.astype(np.float32)

    dinv_cols = np.zeros((N_CORES, 128, NT), np.float32)
    node_grid = (
        np.arange(N_CORES)[:, None, None] * NSHARD
        + np.arange(NT)[None, None, :] * TILE
        + np.arange(128)[None, :, None]
    )
    local = np.arange(NT)[None, None, :] * TILE + np.arange(128)[None, :, None]
    valid = np.broadcast_to(local < NSHARD, node_grid.shape)
    dinv_cols[:] = np.where(valid, dinv[np.where(valid, node_grid, 0)], 0.0)

    xt = np.zeros((PADN_ALL, D_IN), np.float32)
    xs = (np.asarray(x, np.float32) * dinv[:, None]).reshape(N_CORES, NSHARD, D_IN)
    xt.reshape(N_CORES, PADN, D_IN)[:, :NSHARD] = xs
    xt = xt.astype(ml_dtypes.bfloat16)

    meta = dict(nb=nb, chunk_col=chunk_col, grp_nc=grp_nc, grp_base=grp_base,
                grp_b_off=grp_b_off, CHC=CHC, IDXC16=IDXC16)
    return xt, idx16, dstloc, dinv_cols, meta


def _build_program(meta):
    nb = meta["nb"]
    chunk_col = meta["chunk_col"]
    grp_nc = meta["grp_nc"]
    grp_base = meta["grp_base"]
    grp_b_off = meta["grp_b_off"]
    CHC, IDXC16 = meta["CHC"], meta["IDXC16"]

    f32 = mybir.dt.float32
    bf16 = mybir.dt.bfloat16
    i16 = mybir.dt.int16
    nc = bacc.Bacc("TRN2", target_bir_lowering=False, debug=False,
                   num_devices=N_CORES, num_swdge_queues=4)

    xt_in = nc.dram_tensor("xt", [PADN_ALL, D_IN], bf16, kind="ExternalInput").ap()
    w1_in = nc.dram_tensor("W1", [D_IN, D_H], f32, kind="ExternalInput").ap()
    w2_in = nc.dram_tensor("W2", [D_H, D_OUT], f32, kind="ExternalInput").ap()
    b1_in = nc.dram_tensor("b1r", [128, D_H], f32, kind="ExternalInput").ap()
    b2_in = nc.dram_tensor("b2r", [128, D_OUT], f32, kind="ExternalInput").ap()
    id_in = nc.dram_tensor("ident", [128, 128], f32, kind="ExternalInput").ap()
    io_in = nc.dram_tensor("iota", [128, 128], f32, kind="ExternalInput").ap()
    dv_in = nc.dram_tensor("dinv_cols", [128, NT], f32, kind="ExternalInput").ap()
    ix_in = nc.dram_tensor("idx16", [128, IDXC16], i16, kind="ExternalInput").ap()
    dl_in = nc.dram_tensor("dstloc", [128, CHC], f32, kind="ExternalInput").ap()
    out_t = nc.dram_tensor("out", [PADN, D_OUT], f32, kind="ExternalOutput").ap()

    rg = [list(range(N_CORES))]

    with tile.TileContext(nc) as tc:
        with tc.tile_pool(name="const", bufs=1) as constp, \
             tc.tile_pool(name="dram", bufs=1, space="DRAM") as dram, \
             tc.tile_pool(name="agg", bufs=2, space="PSUM") as aggp, \
             tc.tile_pool(name="tp", bufs=2, space="PSUM") as tpp, \
             tc.tile_pool(name="proj", bufs=2, space="PSUM") as projp, \
             tc.tile_pool(name="gat", bufs=3) as gatp, \
             tc.tile_pool(name="sel", bufs=2) as selp, \
             tc.tile_pool(name="sb", bufs=3) as sb:

            nc.gpsimd.load_library(mlp)

            w1 = constp.tile([D_IN, D_H], f32)
            nc.sync.dma_start(w1[:], w1_in[:])
            w2 = constp.tile([D_H, D_OUT], f32)
            nc.sync.dma_start(w2[:], w2_in[:])
            b1r = constp.tile([128, D_H], f32)
            nc.sync.dma_start(b1r[:], b1_in[:])
            b2r = constp.tile([128, D_OUT], f32)
            nc.sync.dma_start(b2r[:], b2_in[:])
            ident = constp.tile([128, 128], f32)
            nc.sync.dma_start(ident[:], id_in[:])
            iota = constp.tile([128, 128], f32)
            nc.sync.dma_start(iota[:], io_in[:])
            dvc = constp.tile([128, NT], f32)
            nc.sync.dma_start(dvc[:], dv_in[:])
            # idx + dstloc resident for both layers
            ixall = constp.tile([128, IDXC16], i16)
            nc.sync.dma_start(ixall[:], ix_in[:])
            dlall = constp.tile([128, CHC], f32)
            nc.sync.dma_start(dlall[:], dl_in[:])

            t2_shard = dram.tile([PADN, D_IN], bf16)
            t2_full = dram.tile([PADN_ALL, D_IN], bf16)

            def edge_pass(table_ap, width, layer):
                """table_ap: [PADN_ALL, 128] bf16 AP. width: 128 or 64."""
                tb0 = table_ap[BIAS[0]:B0_ROWS, :]
                tb1 = table_ap[BIAS[1]:PADN_ALL, :]
                for g in range(N_GROUP):
                    base = int(grp_base[g])
                    ncg = int(grp_nc[g])
                    G = gatp.tile([128, ncg, D_IN], bf16, tag="G")
                    for b in range(N_BUCKET):
                        off = int(grp_b_off[g, b])
                        ncb = int(grp_b_off[g, b + 1]) - off
                        nidx = ncb * 128
                        nc.gpsimd.dma_gather(
                            G[:, off:off + ncb, :],
                            tb0 if b == 0 else tb1,
                            ixall[:, (base + off) * 8:(base + off + ncb) * 8],
                            nidx, nidx, D_IN,
                            single_packet=False,
                            queue_num=(2 * g + b) % 4,
                        )
                    for t in range(g * GROUP, (g + 1) * GROUP):
                        nb0 = int(nb[t, 0])
                        nb1 = int(nb[t, 1])
                        nct = nb0 + nb1
                        l0 = int(chunk_col[t, 0]) - base
                        l1 = int(chunk_col[t, 1]) - base
                        S0 = selp.tile([128, nb0, 128], bf16, tag="S0")
                        nc.vector.tensor_tensor(
                            out=S0[:],
                            in0=dlall[:, base + l0:base + l0 + nb0]
                                .to_broadcast([128, nb0, 128]),
                            in1=iota[:].unsqueeze(1).to_broadcast([128, nb0, 128]),
                            op=mybir.AluOpType.is_equal,
                        )
                        S1 = selp.tile([128, nb1, 128], bf16, tag="S1")
                        nc.vector.tensor_tensor(
                            out=S1[:],
                            in0=dlall[:, base + l1:base + l1 + nb1]
                                .to_broadcast([128, nb1, 128]),
                            in1=iota[:].unsqueeze(1).to_broadcast([128, nb1, 128]),
                            op=mybir.AluOpType.is_equal,
                        )
                        # aggT[f, d] = sum_e G[e, f] * S[e, d]  (G stationary,
                        # S moving) -> agg arrives pre-transposed for the
                        # feature-contraction projection matmul: no transpose.
                        aggT = aggp.tile([width, 128], f32, tag="agg")
                        for i in range(nct):
                            S = S0[:, i, :] if i < nb0 else S1[:, i - nb0, :]
                            gc = (l0 + i) if i < nb0 else (l1 + i - nb0)
                            nc.tensor.matmul(
                                aggT[:], lhsT=G[:, gc, 0:width],
                                rhs=S,
                                start=(i == 0), stop=(i == nct - 1),
                            )
                        aggT_sb = sb.tile([width, 128], f32, tag="e1")
                        nc.scalar.copy(aggT_sb[:], aggT[:])
                        if layer == 1:
                            # t2 = dinv*relu(dinv*(agg @ W1) + b1)
                            proj = projp.tile([128, D_H], f32, tag="proj")
                            nc.tensor.matmul(proj[:], lhsT=aggT_sb[:], rhs=w1[:],
                                             start=True, stop=True)
                            hv = sb.tile([128, D_H], f32, tag="e3")
                            nc.scalar.activation(
                                hv[:], proj[:],
                                mybir.ActivationFunctionType.Copy,
                                scale=dvc[:, t:t + 1])
                            hb = sb.tile([128, D_H], f32, tag="e4")
                            nc.vector.tensor_add(hb[:], hv[:], b1r[:])
                            hr = sb.tile([128, D_H], f32, tag="e5")
                            nc.scalar.activation(
                                hr[:], hb[:], mybir.ActivationFunctionType.Relu)
                            t2t = sb.tile([128, D_H], bf16, tag="e6")
                            nc.scalar.activation(
                                t2t[:], hr[:],
                                mybir.ActivationFunctionType.Copy,
                                scale=dvc[:, t:t + 1])
                            nc.sync.dma_start(
                                t2_shard[t * 128:(t + 1) * 128, 0:D_H], t2t[:])
                        else:
                            # out = dinv*(agg @ W2) + b2
                            o_ps = projp.tile([128, D_OUT], f32, tag="proj")
                            nc.tensor.matmul(o_ps[:], lhsT=aggT_sb[:], rhs=w2[:],
                                             start=True, stop=True)
                            ov = sb.tile([128, D_OUT], f32, tag="e3")
                            nc.scalar.activation(
                                ov[:], o_ps[:],
                                mybir.ActivationFunctionType.Copy,
                                scale=dvc[:, t:t + 1])
                            ob = sb.tile([128, D_OUT], f32, tag="e4")
                            nc.vector.tensor_add(ob[:], ov[:], b2r[:])
                            nc.sync.dma_start(
                                out_t[t * 128:(t + 1) * 128, :], ob[:])

            edge_pass(xt_in, 128, layer=1)

            nc.gpsimd.collective_compute(
                "AllGather", mybir.AluOpType.bypass,
                ins=[t2_shard.opt()], outs=[t2_full.opt()],
                replica_groups=rg,
            )

            edge_pass(t2_full[:], D_H, layer=2)

    nc.compile()
    return nc


def kernel(x, edge_index, W1, b1, W2, b2):
    global LAST_RESULT
    x = np.asarray(x, np.float32)
    W1 = np.asarray(W1, np.float32)
    W2 = np.asarray(W2, np.float32)
    b1 = np.asarray(b1, np.float32)
    b2 = np.asarray(b2, np.float32)

    xt, idx16, dstloc, dinv_cols, meta = _host_prep(x, edge_index)
    nc = _build_program(meta)

    ident = np.eye(128, dtype=np.float32)
    iota = np.tile(np.arange(128, dtype=np.float32), (128, 1))
    b1r = np.tile(b1[None, :], (128, 1)).astype(np.float32)
    b2r = np.tile(b2[None, :], (128, 1)).astype(np.float32)

    in_maps = []
    for k in range(N_CORES):
        in_maps.append({
            "xt": xt,
            "W1": W1, "W2": W2, "b1r": b1r, "b2r": b2r,
            "ident": ident, "iota": iota,
            "dinv_cols": dinv_cols[k],
            "idx16": idx16[k],
            "dstloc": dstloc[k],
        })

    trace = bool(os.environ.get("BASS_TRACE"))
    res = bass_utils.run_bass_kernel_spmd(
        nc, in_maps, core_ids=list(range(N_CORES)), trace=trace)
    LAST_RESULT = res

    out = np.empty((N_NODES, D_OUT), np.float32)
    for k in range(N_CORES):
        out[k * NSHARD:(k + 1) * NSHARD] = res.results[k]["out"][:NSHARD]
    return out


# revision 11
# speedup vs baseline: 2.9393x; 1.1829x over previous
"""2-layer GCN on 8 Trainium2 NeuronCores — aggregate-then-project.

Nodes are range-sharded across 8 cores (dst parallel). Both GCN layers are
computed as: gather source rows from a bf16 feature table (dma_gather with
2 src buckets, biased signed-int16 indices), segment-sum via one-hot bf16
matmuls into PSUM, then per-dst-tile projection:

  layer1 table = dinv*x (host-precomputed, full -> no collective needed)
      aggx[d]  = sum_{s->d} table1[s]          (self-loops in edge list)
      t2[d]    = dinv[d] * relu(dinv[d]*(aggx[d] @ W1) + b1)
  AllGather t2 -> table2
      out[d]   = dinv[d]*(agg2[d] @ W2) + b2

Gathers are batched per group of 7 dst tiles (2 per group: one per bucket)
to amortize the ~5.5us fixed cost per dma_gather instruction.
"""
import os
import sys

sys.path.insert(0, "/opt/trn_rl_repo")

import numpy as np
import ml_dtypes

import concourse.bass as bass
import concourse.bacc as bacc
import concourse.tile as tile
import concourse.mybir as mybir
from concourse import bass_utils
from concourse.library_config import mlp

N_CORES = 8
N_NODES = 100000
D_IN, D_H, D_OUT = 128, 64, 64
NSHARD = N_NODES // N_CORES          # 12500
TILE = 128
NT = (NSHARD + TILE - 1) // TILE     # 98
PADN = NT * TILE                     # 12544
PADN_ALL = N_CORES * PADN            # 100352
N_BUCKET = 2
B0_ROWS = 50176
BIAS = (32768, B0_ROWS + 32768)
GROUP = 7
N_GROUP = NT // GROUP                # 14

LAST_RESULT = None


def _host_prep(x, edge_index):
    src = np.asarray(edge_index[0], dtype=np.int64)
    dst = np.asarray(edge_index[1], dtype=np.int64)
    n = N_NODES

    deg = np.bincount(dst, minlength=n).astype(np.float64) + 1.0
    dinv = (1.0 / np.sqrt(deg)).astype(np.float32)

    loops = np.arange(n, dtype=np.int64)
    s_all = np.concatenate([src, loops])
    d_all = np.concatenate([dst, loops])

    core = d_all // NSHARD
    drem = d_all % NSHARD
    t_id = drem // TILE
    dloc = drem % TILE
    gsrc = (s_all // NSHARD) * PADN + (s_all % NSHARD)
    bkt = (gsrc >= B0_ROWS).astype(np.int64)

    key = (core * NT + t_id) * N_BUCKET + bkt
    order = np.argsort(key, kind="stable")
    key_s = key[order]
    gsrc_s = gsrc[order]
    dloc_s = dloc[order]

    ngroups = N_CORES * NT * N_BUCKET
    counts = np.bincount(key_s, minlength=ngroups).reshape(N_CORES, NT, N_BUCKET)
    # floor+1 chunks: every (t,b) ends with >=1 pad slot on every core, so the
    # gather ucode's trailing-negative trim can never drop a real edge.
    nb = counts.max(axis=0) // 128 + 1                  # [NT, N_BUCKET]

    # slot layout: per group of GROUP tiles: all b0 chunks (tile-major), then
    # all b1 chunks -> one contiguous gather dst region per (group, bucket).
    chunk_col = np.zeros((NT, N_BUCKET), np.int64)
    grp_nc = np.zeros(N_GROUP, np.int64)
    grp_base = np.zeros(N_GROUP, np.int64)
    grp_b_off = np.zeros((N_GROUP, N_BUCKET + 1), np.int64)
    pos = 0
    for g in range(N_GROUP):
        t0 = g * GROUP
        grp_base[g] = pos
        for b in range(N_BUCKET):
            grp_b_off[g, b] = pos - grp_base[g]
            for t in range(t0, t0 + GROUP):
                chunk_col[t, b] = pos
                pos += nb[t, b]
        grp_nc[g] = pos - grp_base[g]
        grp_b_off[g, N_BUCKET] = grp_nc[g]
    CHC = pos
    IDXC16 = CHC * 8

    grp_start = np.zeros(ngroups + 1, np.int64)
    np.cumsum(counts.reshape(-1), out=grp_start[1:])
    rank = np.arange(key_s.shape[0], dtype=np.int64) - grp_start[key_s]

    core_s = key_s // (NT * N_BUCKET)
    tb = key_s % (NT * N_BUCKET)
    t_s = tb // N_BUCKET
    b_s = tb % N_BUCKET

    slot = chunk_col[t_s, b_s] * 128 + rank
    ccol = slot // 128
    cpart = slot % 128

    idx_val = gsrc_s - np.where(b_s == 0, BIAS[0], BIAS[1])
    assert idx_val.min() >= -32768 and idx_val.max() <= 32767

    idx16_16 = np.zeros((N_CORES, 16, IDXC16), np.int16)
    idx16_16[core_s, slot % 16, slot // 16] = idx_val.astype(np.int16)
    idx16 = np.tile(idx16_16, (1, 8, 1))                # [cores, 128, IDXC16]

    dstloc = np.full((N_CORES, 128, CHC), 999.0, np.float32)
    dstloc[core_s, cpart, ccol] = dloc_s.astype(np.float32)

    dinv_cols = np.zeros((N_CORES, 128, NT), np.float32)
    node_grid = (
        np.arange(N_CORES)[:, None, None] * NSHARD
        + np.arange(NT)[None, None, :] * TILE
        + np.arange(128)[None, :, None]
    )
    local = np.arange(NT)[None, None, :] * TILE + np.arange(128)[None, :, None]
    valid = np.broadcast_to(local < NSHARD, node_grid.shape)
    dinv_cols[:] = np.where(valid, dinv[np.where(valid, node_grid, 0)], 0.0)

    xt = np.zeros((PADN_ALL, D_IN), np.float32)
    xs = (np.asarray(x, np.float32) * dinv[:, None]).reshape(N_CORES, NSHARD, D_IN)
    xt.reshape(N_CORES, PADN, D_IN)[:, :NSHARD] = xs
    xt = xt.astype(ml_dtypes.bfloat16)

    meta = dict(nb=nb, chunk_col=chunk_col, grp_nc=grp_nc, grp_base=grp_base,
                grp_b_off=grp_b_off, CHC=CHC, IDXC16=IDXC16)
    return xt, idx16, dstloc, dinv_cols, meta


def _build_program(meta):
    nb = meta["nb"]
    chunk_col = meta["chunk_col"]
    grp_nc = meta["grp_nc"]
    grp_base = meta["grp_base"]
    grp_b_off = meta["grp_b_off"]
    CHC, IDXC16 = meta["CHC"], meta["IDXC16"]

    f32 = mybir.dt.float32
    bf16 = mybir.dt.bfloat16
    i16 = mybir.dt.int16
    nc = bacc.Bacc("TRN2", target_bir_lowering=False, debug=False,
                   num_devices=N_CORES, num_swdge_queues=4)

    xt_in = nc.dram_tensor("xt", [PADN_ALL, D_IN], bf16, kind="ExternalInput").ap()
    w1_in = nc.dram_tensor("W1", [D_IN, D_H], f32, kind="ExternalInput").ap()
    w2_in = nc.dram_tensor("W2", [D_H, D_OUT], f32, kind="ExternalInput").ap()
    b1_in = nc.dram_tensor("b1r", [128, D_H], f32, kind="ExternalInput").ap()
    b2_in = nc.dram_tensor("b2r", [128, D_OUT], f32, kind="ExternalInput").ap()
    id_in = nc.dram_tensor("ident", [128, 128], f32, kind="ExternalInput").ap()
    io_in = nc.dram_tensor("iota", [128, 128], f32, kind="ExternalInput").ap()
    dv_in = nc.dram_tensor("dinv_cols", [128, NT], f32, kind="ExternalInput").ap()
    ix_in = nc.dram_tensor("idx16", [128, IDXC16], i16, kind="ExternalInput").ap()
    dl_in = nc.dram_tensor("dstloc", [128, CHC], f32, kind="ExternalInput").ap()
    out_t = nc.dram_tensor("out", [PADN, D_OUT], f32, kind="ExternalOutput").ap()

    rg = [list(range(N_CORES))]

    with tile.TileContext(nc) as tc:
        with tc.tile_pool(name="const", bufs=1) as constp, \
             tc.tile_pool(name="dram", bufs=1, space="DRAM") as dram, \
             tc.tile_pool(name="agg", bufs=2, space="PSUM") as aggp, \
             tc.tile_pool(name="tp", bufs=2, space="PSUM") as tpp, \
             tc.tile_pool(name="proj", bufs=2, space="PSUM") as projp, \
             tc.tile_pool(name="gat", bufs=3) as gatp, \
             tc.tile_pool(name="sel", bufs=2) as selp, \
             tc.tile_pool(name="sb", bufs=3) as sb:

            nc.gpsimd.load_library(mlp)

            w1 = constp.tile([D_IN, D_H], f32)
            nc.sync.dma_start(w1[:], w1_in[:])
            w2 = constp.tile([D_H, D_OUT], f32)
            nc.sync.dma_start(w2[:], w2_in[:])
            b1r = constp.tile([128, D_H], f32)
            nc.sync.dma_start(b1r[:], b1_in[:])
            b2r = constp.tile([128, D_OUT], f32)
            nc.sync.dma_start(b2r[:], b2_in[:])
            ident = constp.tile([128, 128], f32)
            nc.sync.dma_start(ident[:], id_in[:])
            iota = constp.tile([128, 128], f32)
            nc.sync.dma_start(iota[:], io_in[:])
            dvc = constp.tile([128, NT], f32)
            nc.sync.dma_start(dvc[:], dv_in[:])
            # idx + dstloc resident for both layers
            ixall = constp.tile([128, IDXC16], i16)
            nc.sync.dma_start(ixall[:], ix_in[:])
            dlall = constp.tile([128, CHC], f32)
            nc.sync.dma_start(dlall[:], dl_in[:])

            t2_shard = dram.tile([PADN, D_IN], bf16)
            t2_full = dram.tile([PADN_ALL, D_IN], bf16)

            def edge_pass(table_ap, width, layer):
                """table_ap: [PADN_ALL, 128] bf16 AP. width: 128 or 64."""
                tb0 = table_ap[BIAS[0]:B0_ROWS, :]
                tb1 = table_ap[BIAS[1]:PADN_ALL, :]
                for g in range(N_GROUP):
                    base = int(grp_base[g])
                    ncg = int(grp_nc[g])
                    G = gatp.tile([128, ncg, D_IN], bf16, tag="G")
                    # 4 sub-gathers per group (one per SWDGE queue): each
                    # bucket's chunk range split at a tile boundary so every
                    # sub-gather still ends in pad slots (trailing-trim safe).
                    qn = 0
                    for b in range(N_BUCKET):
                        t0 = g * GROUP
                        tmid = t0 + GROUP // 2
                        lo = int(grp_b_off[g, b])
                        mid = int(chunk_col[tmid, b]) - base
                        hi = int(grp_b_off[g, b + 1])
                        for c0, c1 in ((lo, mid), (mid, hi)):
                            ncb = c1 - c0
                            if ncb == 0:
                                continue
                            nidx = ncb * 128
                            nc.gpsimd.dma_gather(
                                G[:, c0:c1, :],
                                tb0 if b == 0 else tb1,
                                ixall[:, (base + c0) * 8:(base + c1) * 8],
                                nidx, nidx, D_IN,
                                single_packet=False,
                                queue_num=qn % 4,
                            )
                            qn += 1
                    for t in range(g * GROUP, (g + 1) * GROUP):
                        nb0 = int(nb[t, 0])
                        nb1 = int(nb[t, 1])
                        nct = nb0 + nb1
                        l0 = int(chunk_col[t, 0]) - base
                        l1 = int(chunk_col[t, 1]) - base
                        S0 = selp.tile([128, nb0, 128], bf16, tag="S0")
                        nc.vector.tensor_tensor(
                            out=S0[:],
                            in0=dlall[:, base + l0:base + l0 + nb0]
                                .to_broadcast([128, nb0, 128]),
                            in1=iota[:].unsqueeze(1).to_broadcast([128, nb0, 128]),
                            op=mybir.AluOpType.is_equal,
                        )
                        S1 = selp.tile([128, nb1, 128], bf16, tag="S1")
                        nc.vector.tensor_tensor(
                            out=S1[:],
                            in0=dlall[:, base + l1:base + l1 + nb1]
                                .to_broadcast([128, nb1, 128]),
                            in1=iota[:].unsqueeze(1).to_broadcast([128, nb1, 128]),
                            op=mybir.AluOpType.is_equal,
                        )
                        # aggT[f, d] = sum_e G[e, f] * S[e, d]  (G stationary,
                        # S moving) -> agg arrives pre-transposed for the
                        # feature-contraction projection matmul: no transpose.
                        aggT = aggp.tile([width, 128], f32, tag="agg")
                        for i in range(nct):
                            S = S0[:, i, :] if i < nb0 else S1[:, i - nb0, :]
                            gc = (l0 + i) if i < nb0 else (l1 + i - nb0)
                            nc.tensor.matmul(
                                aggT[:], lhsT=G[:, gc, 0:width],
                                rhs=S,
                                start=(i == 0), stop=(i == nct - 1),
                            )
                        aggT_sb = sb.tile([width, 128], f32, tag="e1")
                        nc.scalar.copy(aggT_sb[:], aggT[:])
                        if layer == 1:
                            # t2 = dinv*relu(dinv*(agg @ W1) + b1)
                            proj = projp.tile([128, D_H], f32, tag="proj")
                            nc.tensor.matmul(proj[:], lhsT=aggT_sb[:], rhs=w1[:],
                                             start=True, stop=True)
                            hv = sb.tile([128, D_H], f32, tag="e3")
                            nc.scalar.activation(
                                hv[:], proj[:],
                                mybir.ActivationFunctionType.Copy,
                                scale=dvc[:, t:t + 1])
                            hb = sb.tile([128, D_H], f32, tag="e4")
                            nc.vector.tensor_add(hb[:], hv[:], b1r[:])
                            hr = sb.tile([128, D_H], f32, tag="e5")
                            nc.scalar.activation(
                                hr[:], hb[:], mybir.ActivationFunctionType.Relu)
                            t2t = sb.tile([128, D_H], bf16, tag="e6")
                            nc.scalar.activation(
                                t2t[:], hr[:],
                                mybir.ActivationFunctionType.Copy,
                                scale=dvc[:, t:t + 1])
                            nc.sync.dma_start(
                                t2_shard[t * 128:(t + 1) * 128, 0:D_H], t2t[:])
                        else:
                            # out = dinv*(agg @ W2) + b2
                            o_ps = projp.tile([128, D_OUT], f32, tag="proj")
                            nc.tensor.matmul(o_ps[:], lhsT=aggT_sb[:], rhs=w2[:],
                                             start=True, stop=True)
                            ov = sb.tile([128, D_OUT], f32, tag="e3")
                            nc.scalar.activation(
                                ov[:], o_ps[:],
                                mybir.ActivationFunctionType.Copy,
                                scale=dvc[:, t:t + 1])
                            ob = sb.tile([128, D_OUT], f32, tag="e4")
                            nc.vector.tensor_add(ob[:], ov[:], b2r[:])
                            nc.sync.dma_start(
                                out_t[t * 128:(t + 1) * 128, :], ob[:])

            edge_pass(xt_in, 128, layer=1)

            nc.gpsimd.collective_compute(
                "AllGather", mybir.AluOpType.bypass,
                ins=[t2_shard.opt()], outs=[t2_full.opt()],
                replica_groups=rg,
            )

            edge_pass(t2_full[:], D_H, layer=2)

    nc.compile()
    return nc


def kernel(x, edge_index, W1, b1, W2, b2):
    global LAST_RESULT
    x = np.asarray(x, np.float32)
    W1 = np.asarray(W1, np.float32)
    W2 = np.asarray(W2, np.float32)
    b1 = np.asarray(b1, np.float32)
    b2 = np.asarray(b2, np.float32)

    xt, idx16, dstloc, dinv_cols, meta = _host_prep(x, edge_index)
    nc = _build_program(meta)

    ident = np.eye(128, dtype=np.float32)
    iota = np.tile(np.arange(128, dtype=np.float32), (128, 1))
    b1r = np.tile(b1[None, :], (128, 1)).astype(np.float32)
    b2r = np.tile(b2[None, :], (128, 1)).astype(np.float32)

    in_maps = []
    for k in range(N_CORES):
        in_maps.append({
            "xt": xt,
            "W1": W1, "W2": W2, "b1r": b1r, "b2r": b2r,
            "ident": ident, "iota": iota,
            "dinv_cols": dinv_cols[k],
            "idx16": idx16[k],
            "dstloc": dstloc[k],
        })

    trace = bool(os.environ.get("BASS_TRACE"))
    res = bass_utils.run_bass_kernel_spmd(
        nc, in_maps, core_ids=list(range(N_CORES)), trace=trace)
    LAST_RESULT = res

    out = np.empty((N_NODES, D_OUT), np.float32)
    for k in range(N_CORES):
        out[k * NSHARD:(k + 1) * NSHARD] = res.results[k]["out"][:NSHARD]
    return out


# revision 12
# speedup vs baseline: 3.0183x; 1.0269x over previous
"""2-layer GCN on 8 Trainium2 NeuronCores — aggregate-then-project.

Nodes are range-sharded across 8 cores (dst parallel). Both GCN layers are
computed as: gather source rows from a bf16 feature table (dma_gather with
2 src buckets, biased signed-int16 indices), segment-sum via one-hot bf16
matmuls into PSUM, then per-dst-tile projection:

  layer1 table = dinv*x (host-precomputed, full -> no collective needed)
      aggx[d]  = sum_{s->d} table1[s]          (self-loops in edge list)
      t2[d]    = dinv[d] * relu(dinv[d]*(aggx[d] @ W1) + b1)
  AllGather t2 -> table2
      out[d]   = dinv[d]*(agg2[d] @ W2) + b2

Gathers are batched per group of 7 dst tiles (2 per group: one per bucket)
to amortize the ~5.5us fixed cost per dma_gather instruction.
"""
import os
import sys

sys.path.insert(0, "/opt/trn_rl_repo")

import numpy as np
import ml_dtypes

import concourse.bass as bass
import concourse.bacc as bacc
import concourse.tile as tile
import concourse.mybir as mybir
from concourse import bass_utils
from concourse.library_config import mlp

N_CORES = 8
N_NODES = 100000
D_IN, D_H, D_OUT = 128, 64, 64
NSHARD = N_NODES // N_CORES          # 12500
TILE = 128
NT = (NSHARD + TILE - 1) // TILE     # 98
PADN = NT * TILE                     # 12544
PADN_ALL = N_CORES * PADN            # 100352
N_BUCKET = 2
B0_ROWS = 50176
BIAS = (32768, B0_ROWS + 32768)
GROUP = 4
GROUPS = [(t, min(t + GROUP, NT)) for t in range(0, NT, GROUP)]
N_GROUP = len(GROUPS)                # 25 (24x4 tiles + 1x2)

LAST_RESULT = None


def _host_prep(x, edge_index):
    src = np.asarray(edge_index[0], dtype=np.int64)
    dst = np.asarray(edge_index[1], dtype=np.int64)
    n = N_NODES

    deg = np.bincount(dst, minlength=n).astype(np.float64) + 1.0
    dinv = (1.0 / np.sqrt(deg)).astype(np.float32)

    loops = np.arange(n, dtype=np.int64)
    s_all = np.concatenate([src, loops])
    d_all = np.concatenate([dst, loops])

    core = d_all // NSHARD
    drem = d_all % NSHARD
    t_id = drem // TILE
    dloc = drem % TILE
    gsrc = (s_all // NSHARD) * PADN + (s_all % NSHARD)
    bkt = (gsrc >= B0_ROWS).astype(np.int64)

    key = (core * NT + t_id) * N_BUCKET + bkt
    order = np.argsort(key, kind="stable")
    key_s = key[order]
    gsrc_s = gsrc[order]
    dloc_s = dloc[order]

    ngroups = N_CORES * NT * N_BUCKET
    counts = np.bincount(key_s, minlength=ngroups).reshape(N_CORES, NT, N_BUCKET)
    # floor+1 chunks: every (t,b) ends with >=1 pad slot on every core, so the
    # gather ucode's trailing-negative trim can never drop a real edge.
    nb = counts.max(axis=0) // 128 + 1                  # [NT, N_BUCKET]

    # slot layout: per group of GROUP tiles: all b0 chunks (tile-major), then
    # all b1 chunks -> one contiguous gather dst region per (group, bucket).
    chunk_col = np.zeros((NT, N_BUCKET), np.int64)
    grp_nc = np.zeros(N_GROUP, np.int64)
    grp_base = np.zeros(N_GROUP, np.int64)
    grp_b_off = np.zeros((N_GROUP, N_BUCKET + 1), np.int64)
    pos = 0
    for g, (t0, t1) in enumerate(GROUPS):
        grp_base[g] = pos
        for b in range(N_BUCKET):
            grp_b_off[g, b] = pos - grp_base[g]
            for t in range(t0, t1):
                chunk_col[t, b] = pos
                pos += nb[t, b]
        grp_nc[g] = pos - grp_base[g]
        grp_b_off[g, N_BUCKET] = grp_nc[g]
    CHC = pos
    IDXC16 = CHC * 8

    grp_start = np.zeros(ngroups + 1, np.int64)
    np.cumsum(counts.reshape(-1), out=grp_start[1:])
    rank = np.arange(key_s.shape[0], dtype=np.int64) - grp_start[key_s]

    core_s = key_s // (NT * N_BUCKET)
    tb = key_s % (NT * N_BUCKET)
    t_s = tb // N_BUCKET
    b_s = tb % N_BUCKET

    slot = chunk_col[t_s, b_s] * 128 + rank
    ccol = slot // 128
    cpart = slot % 128

    idx_val = gsrc_s - np.where(b_s == 0, BIAS[0], BIAS[1])
    assert idx_val.min() >= -32768 and idx_val.max() <= 32767

    idx16_16 = np.zeros((N_CORES, 16, IDXC16), np.int16)
    idx16_16[core_s, slot % 16, slot // 16] = idx_val.astype(np.int16)
    idx16 = np.tile(idx16_16, (1, 8, 1))                # [cores, 128, IDXC16]

    dstloc = np.full((N_CORES, 128, CHC), 999.0, np.float32)
    dstloc[core_s, cpart, ccol] = dloc_s.astype(np.float32)

    dinv_cols = np.zeros((N_CORES, 128, NT), np.float32)
    node_grid = (
        np.arange(N_CORES)[:, None, None] * NSHARD
        + np.arange(NT)[None, None, :] * TILE
        + np.arange(128)[None, :, None]
    )
    local = np.arange(NT)[None, None, :] * TILE + np.arange(128)[None, :, None]
    valid = np.broadcast_to(local < NSHARD, node_grid.shape)
    dinv_cols[:] = np.where(valid, dinv[np.where(valid, node_grid, 0)], 0.0)

    xt = np.zeros((PADN_ALL, D_IN), np.float32)
    xs = (np.asarray(x, np.float32) * dinv[:, None]).reshape(N_CORES, NSHARD, D_IN)
    xt.reshape(N_CORES, PADN, D_IN)[:, :NSHARD] = xs
    xt = xt.astype(ml_dtypes.bfloat16)

    meta = dict(nb=nb, chunk_col=chunk_col, grp_nc=grp_nc, grp_base=grp_base,
                grp_b_off=grp_b_off, CHC=CHC, IDXC16=IDXC16)
    return xt, idx16, dstloc, dinv_cols, meta


def _build_program(meta):
    nb = meta["nb"]
    chunk_col = meta["chunk_col"]
    grp_nc = meta["grp_nc"]
    grp_base = meta["grp_base"]
    grp_b_off = meta["grp_b_off"]
    CHC, IDXC16 = meta["CHC"], meta["IDXC16"]

    f32 = mybir.dt.float32
    bf16 = mybir.dt.bfloat16
    i16 = mybir.dt.int16
    nc = bacc.Bacc("TRN2", target_bir_lowering=False, debug=False,
                   num_devices=N_CORES, num_swdge_queues=4)

    xt_in = nc.dram_tensor("xt", [PADN_ALL, D_IN], bf16, kind="ExternalInput").ap()
    w1_in = nc.dram_tensor("W1", [D_IN, D_H], f32, kind="ExternalInput").ap()
    w2_in = nc.dram_tensor("W2", [D_H, D_OUT], f32, kind="ExternalInput").ap()
    b1_in = nc.dram_tensor("b1r", [128, D_H], f32, kind="ExternalInput").ap()
    b2_in = nc.dram_tensor("b2r", [128, D_OUT], f32, kind="ExternalInput").ap()
    id_in = nc.dram_tensor("ident", [128, 128], f32, kind="ExternalInput").ap()
    io_in = nc.dram_tensor("iota", [128, 128], f32, kind="ExternalInput").ap()
    dv_in = nc.dram_tensor("dinv_cols", [128, NT], f32, kind="ExternalInput").ap()
    ix_in = nc.dram_tensor("idx16", [128, IDXC16], i16, kind="ExternalInput").ap()
    dl_in = nc.dram_tensor("dstloc", [128, CHC], f32, kind="ExternalInput").ap()
    out_t = nc.dram_tensor("out", [PADN, D_OUT], f32, kind="ExternalOutput").ap()

    rg = [list(range(N_CORES))]

    with tile.TileContext(nc) as tc:
        with tc.tile_pool(name="const", bufs=1) as constp, \
             tc.tile_pool(name="dram", bufs=1, space="DRAM") as dram, \
             tc.tile_pool(name="agg", bufs=2, space="PSUM") as aggp, \
             tc.tile_pool(name="tp", bufs=2, space="PSUM") as tpp, \
             tc.tile_pool(name="proj", bufs=2, space="PSUM") as projp, \
             tc.tile_pool(name="gat", bufs=5) as gatp, \
             tc.tile_pool(name="sel", bufs=2) as selp, \
             tc.tile_pool(name="sb", bufs=3) as sb:

            nc.gpsimd.load_library(mlp)

            w1 = constp.tile([D_IN, D_H], f32)
            nc.sync.dma_start(w1[:], w1_in[:])
            w2 = constp.tile([D_H, D_OUT], f32)
            nc.sync.dma_start(w2[:], w2_in[:])
            b1r = constp.tile([128, D_H], f32)
            nc.sync.dma_start(b1r[:], b1_in[:])
            b2r = constp.tile([128, D_OUT], f32)
            nc.sync.dma_start(b2r[:], b2_in[:])
            ident = constp.tile([128, 128], f32)
            nc.sync.dma_start(ident[:], id_in[:])
            iota = constp.tile([128, 128], f32)
            nc.sync.dma_start(iota[:], io_in[:])
            dvc = constp.tile([128, NT], f32)
            nc.sync.dma_start(dvc[:], dv_in[:])
            # idx + dstloc resident for both layers
            ixall = constp.tile([128, IDXC16], i16)
            nc.sync.dma_start(ixall[:], ix_in[:])
            dlall = constp.tile([128, CHC], f32)
            nc.sync.dma_start(dlall[:], dl_in[:])

            t2_shard = dram.tile([PADN, D_IN], bf16)
            t2_full = dram.tile([PADN_ALL, D_IN], bf16)

            def edge_pass(table_ap, width, layer):
                """table_ap: [PADN_ALL, 128] bf16 AP. width: 128 or 64."""
                tb0 = table_ap[BIAS[0]:B0_ROWS, :]
                tb1 = table_ap[BIAS[1]:PADN_ALL, :]
                for g, (t0g, t1g) in enumerate(GROUPS):
                    base = int(grp_base[g])
                    ncg = int(grp_nc[g])
                    G = gatp.tile([128, ncg, D_IN], bf16, tag="G")
                    # 4 sub-gathers per group (one per SWDGE queue): each
                    # bucket's chunk range split at a tile boundary so every
                    # sub-gather still ends in pad slots (trailing-trim safe).
                    qn = g
                    for b in range(N_BUCKET):
                        tmid = (t0g + t1g) // 2
                        lo = int(grp_b_off[g, b])
                        mid = int(chunk_col[tmid, b]) - base
                        hi = int(grp_b_off[g, b + 1])
                        for c0, c1 in ((lo, mid), (mid, hi)):
                            ncb = c1 - c0
                            if ncb == 0:
                                continue
                            nidx = ncb * 128
                            nc.gpsimd.dma_gather(
                                G[:, c0:c1, :],
                                tb0 if b == 0 else tb1,
                                ixall[:, (base + c0) * 8:(base + c1) * 8],
                                nidx, nidx, D_IN,
                                single_packet=False,
                                queue_num=qn % 4,
                            )
                            qn += 1
                    for t in range(t0g, t1g):
                        nb0 = int(nb[t, 0])
                        nb1 = int(nb[t, 1])
                        nct = nb0 + nb1
                        l0 = int(chunk_col[t, 0]) - base
                        l1 = int(chunk_col[t, 1]) - base
                        S0 = selp.tile([128, nb0, 128], bf16, tag="S0")
                        nc.vector.tensor_tensor(
                            out=S0[:],
                            in0=dlall[:, base + l0:base + l0 + nb0]
                                .to_broadcast([128, nb0, 128]),
                            in1=iota[:].unsqueeze(1).to_broadcast([128, nb0, 128]),
                            op=mybir.AluOpType.is_equal,
                        )
                        S1 = selp.tile([128, nb1, 128], bf16, tag="S1")
                        nc.vector.tensor_tensor(
                            out=S1[:],
                            in0=dlall[:, base + l1:base + l1 + nb1]
                                .to_broadcast([128, nb1, 128]),
                            in1=iota[:].unsqueeze(1).to_broadcast([128, nb1, 128]),
                            op=mybir.AluOpType.is_equal,
                        )
                        # aggT[f, d] = sum_e G[e, f] * S[e, d]  (G stationary,
                        # S moving) -> agg arrives pre-transposed for the
                        # feature-contraction projection matmul: no transpose.
                        aggT = aggp.tile([width, 128], f32, tag="agg")
                        for i in range(nct):
                            S = S0[:, i, :] if i < nb0 else S1[:, i - nb0, :]
                            gc = (l0 + i) if i < nb0 else (l1 + i - nb0)
                            nc.tensor.matmul(
                                aggT[:], lhsT=G[:, gc, 0:width],
                                rhs=S,
                                start=(i == 0), stop=(i == nct - 1),
                            )
                        aggT_sb = sb.tile([width, 128], f32, tag="e1")
                        nc.scalar.copy(aggT_sb[:], aggT[:])
                        if layer == 1:
                            # t2 = dinv*relu(dinv*(agg @ W1) + b1)
                            proj = projp.tile([128, D_H], f32, tag="proj")
                            nc.tensor.matmul(proj[:], lhsT=aggT_sb[:], rhs=w1[:],
                                             start=True, stop=True)
                            hv = sb.tile([128, D_H], f32, tag="e3")
                            nc.scalar.activation(
                                hv[:], proj[:],
                                mybir.ActivationFunctionType.Copy,
                                scale=dvc[:, t:t + 1])
                            hb = sb.tile([128, D_H], f32, tag="e4")
                            nc.vector.tensor_add(hb[:], hv[:], b1r[:])
                            hr = sb.tile([128, D_H], f32, tag="e5")
                            nc.scalar.activation(
                                hr[:], hb[:], mybir.ActivationFunctionType.Relu)
                            t2t = sb.tile([128, D_H], bf16, tag="e6")
                            nc.scalar.activation(
                                t2t[:], hr[:],
                                mybir.ActivationFunctionType.Copy,
                                scale=dvc[:, t:t + 1])
                            nc.sync.dma_start(
                                t2_shard[t * 128:(t + 1) * 128, 0:D_H], t2t[:])
                        else:
                            # out = dinv*(agg @ W2) + b2
                            o_ps = projp.tile([128, D_OUT], f32, tag="proj")
                            nc.tensor.matmul(o_ps[:], lhsT=aggT_sb[:], rhs=w2[:],
                                             start=True, stop=True)
                            ov = sb.tile([128, D_OUT], f32, tag="e3")
                            nc.scalar.activation(
                                ov[:], o_ps[:],
                                mybir.ActivationFunctionType.Copy,
                                scale=dvc[:, t:t + 1])
                            ob = sb.tile([128, D_OUT], f32, tag="e4")
                            nc.vector.tensor_add(ob[:], ov[:], b2r[:])
                            nc.sync.dma_start(
                                out_t[t * 128:(t + 1) * 128, :], ob[:])

            edge_pass(xt_in, 128, layer=1)

            nc.gpsimd.collective_compute(
                "AllGather", mybir.AluOpType.bypass,
                ins=[t2_shard.opt()], outs=[t2_full.opt()],
                replica_groups=rg,
            )

            edge_pass(t2_full[:], D_H, layer=2)

    nc.compile()
    return nc


def kernel(x, edge_index, W1, b1, W2, b2):
    global LAST_RESULT
    x = np.asarray(x, np.float32)
    W1 = np.asarray(W1, np.float32)
    W2 = np.asarray(W2, np.float32)
    b1 = np.asarray(b1, np.float32)
    b2 = np.asarray(b2, np.float32)

    xt, idx16, dstloc, dinv_cols, meta = _host_prep(x, edge_index)
    nc = _build_program(meta)

    ident = np.eye(128, dtype=np.float32)
    iota = np.tile(np.arange(128, dtype=np.float32), (128, 1))
    b1r = np.tile(b1[None, :], (128, 1)).astype(np.float32)
    b2r = np.tile(b2[None, :], (128, 1)).astype(np.float32)

    in_maps = []
    for k in range(N_CORES):
        in_maps.append({
            "xt": xt,
            "W1": W1, "W2": W2, "b1r": b1r, "b2r": b2r,
            "ident": ident, "iota": iota,
            "dinv_cols": dinv_cols[k],
            "idx16": idx16[k],
            "dstloc": dstloc[k],
        })

    trace = bool(os.environ.get("BASS_TRACE"))
    res = bass_utils.run_bass_kernel_spmd(
        nc, in_maps, core_ids=list(range(N_CORES)), trace=trace)
    LAST_RESULT = res

    out = np.empty((N_NODES, D_OUT), np.float32)
    for k in range(N_CORES):
        out[k * NSHARD:(k + 1) * NSHARD] = res.results[k]["out"][:NSHARD]
    return out
